# revision 14
# baseline (speedup 1.0000x reference)
"""Grouped (MoE-routed) GEMM on 8 Trainium2 NeuronCores.

out[m, n] = sum_k lhs[m, k] * rhs[g[m], n, k],  g = clamp(m_indices, 0, G)

Strategy: expert-parallel. Host dispatches rows by m_indices (the
"all-to-all" is a host-side gather since we hold full inputs), core c gets
expert c's rows padded to a common M_pad, plus expert c's weight matrix.
Every core then runs one identical dense GEMM program computing the
transposed output:

    oT[N, M_pad] = B[N, K] @ A[M_pad, K]^T    (bf16 in, fp32 accum, bf16 out)

All operands are pre-laid-out on the host as [128-partition, k-chunk, col]
so every device DMA is one large elementwise-aligned 3D copy — per-trace,
the old per-(k-chunk) loads were DMA-*issue*-bound (~650ns of engine time
per dma_start, 64 loads + 96 stores), starving the PE for ~20us and
tripping a HAM re-throttle.  Now: ~16 load DMAs + 10 store DMAs total.
DMA reality (measured): ~150-185GB/s per HWDGE ring, ~1.5us start and
~1.8us completion-receipt latency per DMA.  So the critical prime (bt
n-tiles 0-1 + the whole 448-wide first at chunk, kc-split so both rings
carry equal bytes) ships first, and the rest of bt arrives as 2-n-tile
strips alternating rings, pacing the first sweep's ~176GB/s consumption;
at chunks 1+ trail behind.  A run of junk matmuls on a scratch tile
bridges the load window so the HAM clock gate (1.2 -> 2.4 GHz) is
released before real work arrives.  Bulk stores ride the otherwise-idle
GpSimd SWDGE path; the final m-chunk is 256 wide and stored in tapered
n-tile groups through the HWDGE rings, so the kernel tail is one tiny
store instead of a bulk-queue drain.  Measured: ~134.8us (PE floor for
m_pad=2112 is 112.6us; preamble ~6.6, prime ~7.8, tail+epilogue ~5).
"""

import numpy as np
import ml_dtypes

K = 1024
N = 2048
G = 8
N_CORES = 8
KP = 128           # SBUF partitions / contraction chunk
KC = K // KP       # 8 k-chunks
NT = N // KP       # 16 stationary n-tiles
MCH = 512          # max moving-operand m-chunk (one PSUM bank of fp32)
MC0 = 448          # first m-chunk: wide enough that the sweep's bt-strip
                   # demand (~176GB/s) stays under the DMA supply rate
N_WARMUP = 12      # junk matmuls bridging the load window (HAM clock gate)

_BUILD_CACHE = {}

# Final-column store groups: emit after this n-tile, covering [lo, hi).
_LAST_STORE_AFTER = {
    3: (0, (0, 4)), 7: (1, (4, 8)), 11: (0, (8, 12)),
    13: (1, (12, 14)), 14: (0, (14, 15)), 15: (1, (15, 16)),
}


def _m_chunks(m_pad):
    """Split m_pad into [~448, near-equal <=512 middles, 256].

    The first chunk is sized so the first sweep's bt demand matches DMA
    supply; the last is small so the kernel tail is one tiny store.  All
    chunks stay >=256 wide so the per-chain LDWEIGHTS stays hidden.
    """
    if m_pad <= 512:
        sizes = [m_pad]
    elif m_pad <= 768:
        sizes = [m_pad - 256, 256]
    elif m_pad < 960:
        sizes = [m_pad - 512, 256, 256]
    else:
        rest = m_pad - MC0 - 256
        n_mid = -(-rest // MCH)
        base = rest // n_mid // 64 * 64
        mids = [base] * n_mid
        extra = rest - base * n_mid
        i = 0
        while extra > 0:
            mids[i] += 64
            extra -= 64
            i = (i + 1) % n_mid
        sizes = [MC0] + mids + [256]
    chunks = []
    m = 0
    for w in sizes:
        chunks.append((m, w))
        m += w
    assert m == m_pad and all(w <= MCH for _, w in chunks)
    return chunks


def _build(m_pad):
    import concourse.mybir as mybir
    import concourse.tile as tile
    from concourse import bacc

    if m_pad in _BUILD_CACHE:
        return _BUILD_CACHE[m_pad]

    nc = bacc.Bacc("TRN2", target_bir_lowering=False, debug=False,
                   num_devices=N_CORES)

    # Host pre-lays everything out partition-major so each DMA below is a
    # single aligned 3D copy with contiguous >=512B last-dim runs.
    at_d = nc.dram_tensor("at", [KP, KC, m_pad], mybir.dt.bfloat16,
                          kind="ExternalInput")
    bt_d = nc.dram_tensor("bt", [KP, KC, N], mybir.dt.bfloat16,
                          kind="ExternalInput")
    o_d = nc.dram_tensor("o", [KP, NT, m_pad], mybir.dt.bfloat16,
                         kind="ExternalOutput")

    chunks = _m_chunks(m_pad)

    with tile.TileContext(nc) as tc:
        with (
            tc.tile_pool(name="ats", bufs=1) as ap,
            tc.tile_pool(name="bts", bufs=1) as bp,
            tc.tile_pool(name="wrm", bufs=1) as wp,
            tc.tile_pool(name="ost", bufs=4) as op,
            tc.tile_pool(name="ps", bufs=8, space="PSUM") as pp,
        ):
            at_s = ap.tile([KP, KC, m_pad], mybir.dt.bfloat16)
            bt_s = bp.tile([KP, KC, N], mybir.dt.bfloat16)

            # PE warmup: junk matmuls (scratch tile) run while input DMAs
            # stream, so the HAM clock gate is released before the first
            # real matmul.
            if N_WARMUP:
                junk = wp.tile([KP, MCH], mybir.dt.bfloat16)
                nc.vector.memset(junk[:], 0.0)
                wps = pp.tile([KP, MCH], mybir.dt.float32, name="wps",
                              tag="ps")
                for _ in range(N_WARMUP):
                    nc.tensor.matmul(wps[:], junk[:, 0:KP], junk[:],
                                     start=True, stop=True)

            # Loads: ordered exactly by first-need.  Per-DMA completion
            # pays ~1.5us start + ~1.8us receipt latency on top of
            # ~150-185GB/s per ring, so the critical prime (bt n-tiles
            # 0-1 + the whole first at chunk, kc-split so both rings
            # carry equal bytes) lands first; the remaining bt arrives
            # as 2-n-tile strips alternating rings, tracking the first
            # sweep's consumption.  at chunks 1+ follow behind.
            (mc0, w0) = chunks[0]
            # kc-phased prime: the first chain eats kc sequentially, so
            # gate each matmul on only its kc slice — six pieces, need-
            # ordered and byte-balanced across the rings.
            nc.scalar.dma_start(at_s[:, 0:2, mc0:mc0 + w0],
                                at_d[:, 0:2, mc0:mc0 + w0])
            nc.sync.dma_start(bt_s[:, 0:4, 0:2 * KP],
                              bt_d[:, 0:4, 0:2 * KP])
            nc.sync.dma_start(at_s[:, 2:4, mc0:mc0 + w0],
                              at_d[:, 2:4, mc0:mc0 + w0])
            nc.scalar.dma_start(bt_s[:, 4:KC, 0:2 * KP],
                                bt_d[:, 4:KC, 0:2 * KP])
            nc.scalar.dma_start(at_s[:, 4:6, mc0:mc0 + w0],
                                at_d[:, 4:6, mc0:mc0 + w0])
            nc.sync.dma_start(at_s[:, 6:KC, mc0:mc0 + w0],
                              at_d[:, 6:KC, mc0:mc0 + w0])
            for s in range(1, 8):       # bt n-tiles 2-15, alternating
                eng = nc.scalar if s % 2 == 1 else nc.sync
                eng.dma_start(bt_s[:, :, s * 2 * KP:(s + 1) * 2 * KP],
                              bt_d[:, :, s * 2 * KP:(s + 1) * 2 * KP])
            for i, (mc, w) in enumerate(chunks[1:]):
                eng = nc.scalar if i % 2 == 0 else nc.sync
                eng.dma_start(at_s[:, :, mc:mc + w], at_d[:, :, mc:mc + w])

            # GEMM: one PSUM accumulation chain per (m-chunk, n-tile),
            # m-chunk-outer so the first column starts on minimal data.
            # Each chunk's 16 result tiles collect in one SBUF tile and
            # leave as a single batched store.
            for ci, (mc, w) in enumerate(chunks):
                last_col = ci == len(chunks) - 1
                ot = op.tile([KP, NT, w], mybir.dt.bfloat16, name="ot")
                for nt in range(NT):
                    p = pp.tile([KP, w], mybir.dt.float32, name="p",
                                tag="ps")
                    for kc in range(KC):
                        nc.tensor.matmul(
                            p[:],
                            bt_s[:, kc, nt * KP:(nt + 1) * KP],
                            at_s[:, kc, mc:mc + w],
                            start=(kc == 0),
                            stop=(kc == KC - 1),
                        )
                    nc.vector.tensor_copy(ot[:, nt, :], p[:])
                    if last_col and nt in _LAST_STORE_AFTER:
                        # Final column: tapered store groups through the
                        # (by now idle) HWDGE rings — the very last store
                        # is a single n-tile, so the kernel tail is one
                        # tiny store instead of a bulk-queue drain.
                        g, (lo, hi) = _LAST_STORE_AFTER[nt]
                        st = nc.sync if g % 2 == 0 else nc.scalar
                        st.dma_start(o_d[:, lo:hi, mc:mc + w],
                                     ot[:, lo:hi, :])
                if not last_col:
                    # Bulk stores ride the otherwise-idle SWDGE path so
                    # the HWDGE rings stay clear for loads.
                    nc.gpsimd.dma_start(o_d[:, :, mc:mc + w], ot[:])

    nc.compile()
    _BUILD_CACHE[m_pad] = nc
    return nc


SEC_CAP = 4096     # max rows one core takes in one launch (bounds SBUF use)


def _shard(m_indices):
    """Dispatch rows to (expert, row-subset) sections, <=8 per launch.

    In the common balanced case this is exactly one section per expert and
    a single launch. If one expert is so heavy that its section exceeds
    SEC_CAP, it is split into multiple sections (and, beyond 8 sections
    total, into multiple launches) so SBUF capacity is never exceeded.
    """
    g = np.where((m_indices >= 0) & (m_indices < G), m_indices, 0)
    rows = [np.nonzero(g == e)[0] for e in range(G)]
    sections = []                        # (expert, row_indices)
    for e in range(G):
        for s in range(0, max(len(rows[e]), 1), SEC_CAP):
            sections.append((e, rows[e][s:s + SEC_CAP]))
    sections.sort(key=lambda s: -len(s[1]))
    launches = [sections[i:i + N_CORES]
                for i in range(0, len(sections), N_CORES)]
    return launches


def _prep_in_maps(lhs, rhs, launch, m_pad):
    in_maps = []
    bt_cache = {}
    for slot in range(N_CORES):
        e, r = launch[slot] if slot < len(launch) else (0, [])
        a = np.zeros((m_pad, K), dtype=ml_dtypes.bfloat16)
        if len(r):
            a[:len(r)] = lhs[r]
        # [m, k] -> [kp, kc, m]: partition-major so device DMAs are
        # elementwise-aligned 3D copies.
        at = a.T.reshape(KC, KP, m_pad).transpose(1, 0, 2)
        if e not in bt_cache:
            bt_cache[e] = np.ascontiguousarray(
                rhs[e].T.reshape(KC, KP, N).transpose(1, 0, 2))
        in_maps.append({
            "at": np.ascontiguousarray(at),
            "bt": bt_cache[e],
        })
    return in_maps


def kernel(lhs, rhs, m_indices):
    from concourse import bass_utils

    lhs = np.asarray(lhs)
    rhs = np.asarray(rhs)
    m_indices = np.asarray(m_indices)
    M = lhs.shape[0]

    out = np.zeros((M, N), dtype=ml_dtypes.bfloat16)
    for launch in _shard(m_indices):
        m_pad = max(-(-max(len(r) for _, r in launch) // 64) * 64, 128)
        nc = _build(m_pad)
        in_maps = _prep_in_maps(lhs, rhs, launch, m_pad)
        res = bass_utils.run_bass_kernel_spmd(
            nc, in_maps, core_ids=list(range(N_CORES)))
        for slot, (e, r) in enumerate(launch):
            if len(r):
                o = res.results[slot]["o"]       # [KP, NT, m_pad]
                oT = o.transpose(1, 0, 2).reshape(N, m_pad)
                out[r] = oT[:, :len(r)].T
    return out


# revision 15
# speedup vs baseline: 1.0133x; 1.0133x over previous
"""Grouped (MoE-routed) GEMM on 8 Trainium2 NeuronCores.

out[m, n] = sum_k lhs[m, k] * rhs[g[m], n, k],  g = clamp(m_indices, 0, G)

Strategy: expert-parallel. Host dispatches rows by m_indices (the
"all-to-all" is a host-side gather since we hold full inputs), core c gets
expert c's rows padded to a common M_pad, plus expert c's weight matrix.
Every core then runs one identical dense GEMM program computing the
transposed output:

    oT[N, M_pad] = B[N, K] @ A[M_pad, K]^T    (bf16 in, fp32 accum, bf16 out)

All operands are pre-laid-out on the host as [128-partition, k-chunk, col]
so every device DMA is one large elementwise-aligned 3D copy — per-trace,
the old per-(k-chunk) loads were DMA-*issue*-bound (~650ns of engine time
per dma_start, 64 loads + 96 stores), starving the PE for ~20us and
tripping a HAM re-throttle.  Now: ~16 load DMAs + 10 store DMAs total.
DMA reality (measured): ~150-185GB/s per HWDGE ring, ~1.5us start and
~1.8us completion-receipt latency per DMA.  So the critical prime (bt
n-tiles 0-1 + the whole 448-wide first at chunk, kc-split so both rings
carry equal bytes) ships first, and the rest of bt arrives as 2-n-tile
strips alternating rings, pacing the first sweep's ~176GB/s consumption;
at chunks 1+ trail behind.  A run of junk matmuls on a scratch tile
bridges the load window so the HAM clock gate (1.2 -> 2.4 GHz) is
released before real work arrives.  Bulk stores ride the otherwise-idle
GpSimd SWDGE path; the final m-chunk is 256 wide and stored in tapered
n-tile groups through the HWDGE rings, so the kernel tail is one tiny
store instead of a bulk-queue drain.  Measured: ~134.8us (PE floor for
m_pad=2112 is 112.6us; preamble ~6.6, prime ~7.8, tail+epilogue ~5).
"""

import numpy as np
import ml_dtypes

K = 1024
N = 2048
G = 8
N_CORES = 8
KP = 128           # SBUF partitions / contraction chunk
KC = K // KP       # 8 k-chunks
NT = N // KP       # 16 stationary n-tiles
MCH = 512          # max moving-operand m-chunk (one PSUM bank of fp32)
MC0 = 448          # first m-chunk: wide enough that the sweep's bt-strip
                   # demand (~176GB/s) stays under the DMA supply rate
N_WARMUP = 22      # junk matmuls bridging the load window (HAM clock gate)

_BUILD_CACHE = {}

# Final-column store groups: emit after this n-tile, covering [lo, hi).
_LAST_STORE_AFTER = {
    3: (0, (0, 4)), 7: (1, (4, 8)), 11: (0, (8, 12)),
    13: (1, (12, 14)), 14: (0, (14, 15)), 15: (1, (15, 16)),
}


def _m_chunks(m_pad):
    """Split m_pad into [~448, near-equal <=512 middles, 256].

    The first chunk is sized so the first sweep's bt demand matches DMA
    supply; the last is small so the kernel tail is one tiny store.  All
    chunks stay >=256 wide so the per-chain LDWEIGHTS stays hidden.
    """
    if m_pad <= 512:
        sizes = [m_pad]
    elif m_pad <= 768:
        sizes = [m_pad - 256, 256]
    elif m_pad < 960:
        sizes = [m_pad - 512, 256, 256]
    else:
        rest = m_pad - MC0 - 256
        n_mid = -(-rest // MCH)
        base = rest // n_mid // 64 * 64
        mids = [base] * n_mid
        extra = rest - base * n_mid
        i = 0
        while extra > 0:
            mids[i] += 64
            extra -= 64
            i = (i + 1) % n_mid
        sizes = [MC0] + mids + [256]
    chunks = []
    m = 0
    for w in sizes:
        chunks.append((m, w))
        m += w
    assert m == m_pad and all(w <= MCH for _, w in chunks)
    return chunks


def _build(m_pad):
    import concourse.mybir as mybir
    import concourse.tile as tile
    from concourse import bacc

    if m_pad in _BUILD_CACHE:
        return _BUILD_CACHE[m_pad]

    nc = bacc.Bacc("TRN2", target_bir_lowering=False, debug=False,
                   num_devices=N_CORES)

    # Host pre-lays everything out partition-major so each DMA below is a
    # single aligned 3D copy with contiguous >=512B last-dim runs.
    at_d = nc.dram_tensor("at", [KP, KC, m_pad], mybir.dt.bfloat16,
                          kind="ExternalInput")
    bt_d = nc.dram_tensor("bt", [KP, KC, N], mybir.dt.bfloat16,
                          kind="ExternalInput")
    o_d = nc.dram_tensor("o", [KP, NT, m_pad], mybir.dt.bfloat16,
                         kind="ExternalOutput")

    chunks = _m_chunks(m_pad)

    with tile.TileContext(nc) as tc:
        with (
            tc.tile_pool(name="ats", bufs=1) as ap,
            tc.tile_pool(name="bts", bufs=1) as bp,
            tc.tile_pool(name="wrm", bufs=1) as wp,
            tc.tile_pool(name="ost", bufs=4) as op,
            tc.tile_pool(name="ps", bufs=8, space="PSUM") as pp,
        ):
            at_s = ap.tile([KP, KC, m_pad], mybir.dt.bfloat16)
            bt_s = bp.tile([KP, KC, N], mybir.dt.bfloat16)

            # PE warmup: junk matmuls (scratch tile) run while input DMAs
            # stream, so the HAM clock gate is released before the first
            # real matmul.
            if N_WARMUP:
                junk = wp.tile([KP, MCH], mybir.dt.bfloat16)
                nc.vector.memset(junk[:], 0.0)
                wps = pp.tile([KP, MCH], mybir.dt.float32, name="wps",
                              tag="ps")
                for _ in range(N_WARMUP):
                    nc.tensor.matmul(wps[:], junk[:, 0:KP], junk[:],
                                     start=True, stop=True)

            # Loads: ordered exactly by first-need.  Per-DMA completion
            # pays ~1.5us start + ~1.8us receipt latency on top of
            # ~150-185GB/s per ring, so the critical prime (bt n-tiles
            # 0-1 + the whole first at chunk, kc-split so both rings
            # carry equal bytes) lands first; the remaining bt arrives
            # as 2-n-tile strips alternating rings, tracking the first
            # sweep's consumption.  at chunks 1+ follow behind.
            (mc0, w0) = chunks[0]
            nc.sync.dma_start(bt_s[:, :, 0:2 * KP], bt_d[:, :, 0:2 * KP])
            nc.sync.dma_start(at_s[:, 0:2, mc0:mc0 + w0],
                              at_d[:, 0:2, mc0:mc0 + w0])
            nc.scalar.dma_start(at_s[:, 2:KC, mc0:mc0 + w0],
                                at_d[:, 2:KC, mc0:mc0 + w0])
            for s in range(1, 8):       # bt n-tiles 2-15, alternating
                eng = nc.scalar if s % 2 == 1 else nc.sync
                eng.dma_start(bt_s[:, :, s * 2 * KP:(s + 1) * 2 * KP],
                              bt_d[:, :, s * 2 * KP:(s + 1) * 2 * KP])
            for i, (mc, w) in enumerate(chunks[1:]):
                eng = nc.scalar if i % 2 == 0 else nc.sync
                eng.dma_start(at_s[:, :, mc:mc + w], at_d[:, :, mc:mc + w])

            # GEMM: one PSUM accumulation chain per (m-chunk, n-tile),
            # m-chunk-outer so the first column starts on minimal data.
            # Each chunk's 16 result tiles collect in one SBUF tile and
            # leave as a single batched store.
            for ci, (mc, w) in enumerate(chunks):
                last_col = ci == len(chunks) - 1
                ot = op.tile([KP, NT, w], mybir.dt.bfloat16, name="ot")
                for nt in range(NT):
                    p = pp.tile([KP, w], mybir.dt.float32, name="p",
                                tag="ps")
                    for kc in range(KC):
                        nc.tensor.matmul(
                            p[:],
                            bt_s[:, kc, nt * KP:(nt + 1) * KP],
                            at_s[:, kc, mc:mc + w],
                            start=(kc == 0),
                            stop=(kc == KC - 1),
                        )
                    nc.vector.tensor_copy(ot[:, nt, :], p[:])
                    if last_col and nt in _LAST_STORE_AFTER:
                        # Final column: tapered store groups through the
                        # (by now idle) HWDGE rings — the very last store
                        # is a single n-tile, so the kernel tail is one
                        # tiny store instead of a bulk-queue drain.
                        g, (lo, hi) = _LAST_STORE_AFTER[nt]
                        st = nc.sync if g % 2 == 0 else nc.scalar
                        st.dma_start(o_d[:, lo:hi, mc:mc + w],
                                     ot[:, lo:hi, :])
                if not last_col:
                    # Bulk stores ride the otherwise-idle SWDGE path so
                    # the HWDGE rings stay clear for loads.
                    nc.gpsimd.dma_start(o_d[:, :, mc:mc + w], ot[:])

    nc.compile()
    _BUILD_CACHE[m_pad] = nc
    return nc


SEC_CAP = 4096     # max rows one core takes in one launch (bounds SBUF use)


def _shard(m_indices):
    """Dispatch rows to (expert, row-subset) sections, <=8 per launch.

    In the common balanced case this is exactly one section per expert and
    a single launch. If one expert is so heavy that its section exceeds
    SEC_CAP, it is split into multiple sections (and, beyond 8 sections
    total, into multiple launches) so SBUF capacity is never exceeded.
    """
    g = np.where((m_indices >= 0) & (m_indices < G), m_indices, 0)
    rows = [np.nonzero(g == e)[0] for e in range(G)]
    sections = []                        # (expert, row_indices)
    for e in range(G):
        for s in range(0, max(len(rows[e]), 1), SEC_CAP):
            sections.append((e, rows[e][s:s + SEC_CAP]))
    sections.sort(key=lambda s: -len(s[1]))
    launches = [sections[i:i + N_CORES]
                for i in range(0, len(sections), N_CORES)]
    return launches


def _prep_in_maps(lhs, rhs, launch, m_pad):
    in_maps = []
    bt_cache = {}
    for slot in range(N_CORES):
        e, r = launch[slot] if slot < len(launch) else (0, [])
        a = np.zeros((m_pad, K), dtype=ml_dtypes.bfloat16)
        if len(r):
            a[:len(r)] = lhs[r]
        # [m, k] -> [kp, kc, m]: partition-major so device DMAs are
        # elementwise-aligned 3D copies.
        at = a.T.reshape(KC, KP, m_pad).transpose(1, 0, 2)
        if e not in bt_cache:
            bt_cache[e] = np.ascontiguousarray(
                rhs[e].T.reshape(KC, KP, N).transpose(1, 0, 2))
        in_maps.append({
            "at": np.ascontiguousarray(at),
            "bt": bt_cache[e],
        })
    return in_maps


def kernel(lhs, rhs, m_indices):
    from concourse import bass_utils

    lhs = np.asarray(lhs)
    rhs = np.asarray(rhs)
    m_indices = np.asarray(m_indices)
    M = lhs.shape[0]

    out = np.zeros((M, N), dtype=ml_dtypes.bfloat16)
    for launch in _shard(m_indices):
        m_pad = max(-(-max(len(r) for _, r in launch) // 64) * 64, 128)
        nc = _build(m_pad)
        in_maps = _prep_in_maps(lhs, rhs, launch, m_pad)
        res = bass_utils.run_bass_kernel_spmd(
            nc, in_maps, core_ids=list(range(N_CORES)))
        for slot, (e, r) in enumerate(launch):
            if len(r):
                o = res.results[slot]["o"]       # [KP, NT, m_pad]
                oT = o.transpose(1, 0, 2).reshape(N, m_pad)
                out[r] = oT[:, :len(r)].T
    return out
